# revision 2
# baseline (speedup 1.0000x reference)
"""GAT (2-layer, PyG-style) on 8 Trainium2 NeuronCores — v2.

Design vs baseline:
  - Layer-1 dense phase is REPLICATED on every core (PE is cheap), writing a
    local bf16 table [R, 384] = [h 256 | as 8 | pad], so the 64MB table-1
    AllGather disappears entirely.
  - All gather tables and PE matmuls in bf16 (2x PE rate, ~2x less HBM).
  - One dma_gather per (dst-block, half) instead of 6-tile chunks (amortizes
    the ~1us SWDGE fixed cost); descriptor ring enlarged.
  - One-hot "St" (dst-major) built via rank-1 PE broadcast of host-supplied
    dlocT + a single is_equal per 512-col chunk (no per-tile PE transposes).
  - X is fed host-pretransposed+interleaved so the dense phases do straight
    matmuls with no on-device transposes.
  - Node rows are laid out core-major (r = core*NPAD + local) for BOTH
    tables, so one int16 index set serves both layers.
"""
import sys
sys.path.insert(0, "/opt/trn_rl_repo")

import numpy as np
import concourse.bass as bass
import concourse.bacc as bacc
import concourse.mybir as mybir
from concourse.tile import TileContext
from concourse.bass_utils import run_bass_kernel_spmd

F32 = mybir.dt.float32
BF16 = mybir.dt.bfloat16
I16 = mybir.dt.int16

P = 128
NCORES = 8
LEAKY = 0.2
EPS = 1e-16


class Cfg:
    def __init__(self, N, E, IN_CH=256, HID=256, OUT_CH=64, H1=8):
        self.N, self.E = N, E
        self.IN_CH, self.HID, self.OUT_CH, self.H1 = IN_CH, HID, OUT_CH, H1
        self.C1 = HID // H1
        self.ND = N // NCORES                    # dst nodes per core
        self.NB = (self.ND + P - 1) // P         # dst blocks per core
        self.NPAD = self.NB * P                  # padded shard rows
        self.R = NCORES * self.NPAD              # global table rows
        self.RB = NCORES * self.NB               # global table blocks
        self.HALF = self.R // 2                  # int16 gather half split
        assert self.HALF % P == 0
        assert self.HALF < 32768 and (self.R - self.HALF) < 32768
        self.TW1 = 384                           # table1 row width bf16 (264 used)
        self.U1 = HID + H1                       # 264
        self.TW2 = 128                           # table2 row width bf16 (65 used)
        self.U2 = OUT_CH + 1                     # 65 = [g 64 | as2]


CFG_FULL = Cfg(N=50000, E=800000)


# ---------------------------------------------------------------- host side
def _node_row(cfg, n):
    # core-major padded row layout, same for table1 and table2
    return (n // cfg.ND) * cfg.NPAD + (n % cfg.ND)


def preprocess_graph(cfg, edge_index):
    """Per-core wrapped int16 gather indices, dloc (edge-major) and dlocT
    (flat, for the St broadcast), plus shared per-block tile counts."""
    src = np.concatenate([edge_index[0], np.arange(cfg.N, dtype=np.int64)])
    dst = np.concatenate([edge_index[1], np.arange(cfg.N, dtype=np.int64)])
    r_src = _node_row(cfg, src)

    core = dst // cfg.ND
    dst_local = dst - core * cfg.ND
    blk = dst_local // P
    dloc = dst_local % P
    in_a = r_src < cfg.HALF

    NB = cfg.NB
    counts = np.zeros((NCORES, NB, 2), dtype=np.int64)
    np.add.at(counts, (core, blk, (~in_a).astype(np.int64)), 1)
    TA = np.maximum(1, (counts[:, :, 0].max(0) + P - 1) // P)
    TB = np.maximum(1, (counts[:, :, 1].max(0) + P - 1) // P)

    order = np.lexsort((in_a * -1, blk, core))  # by core, block, half (A first)
    rsrc_s, core_s, blk_s, dloc_s, ina_s = (
        r_src[order], core[order], blk[order], dloc[order], in_a[order])

    Tsum = int((TA + TB).sum())
    idx16 = []   # per core: [128, 8 * Tsum] int16
    dlocf = []   # per core: [128, Tsum] bf16-able float
    dloct = []   # per core: [1, Tsum*128] float
    for c in range(NCORES):
        iw = np.zeros((P, 8 * Tsum), dtype=np.int16)
        dw = np.full((P, Tsum), -1.0, dtype=np.float32)
        dt_ = np.full((1, Tsum * P), -1.0, dtype=np.float32)
        csel = core_s == c
        col0 = 0
        for b in range(NB):
            bsel = csel & (blk_s == b)
            for half, T in ((0, int(TA[b])), (1, int(TB[b]))):
                hsel = bsel & (ina_s == (half == 0))
                rr = rsrc_s[hsel] - (0 if half == 0 else cfg.HALF)
                dd = dloc_s[hsel]
                S = T * P
                assert len(rr) <= S
                idx = np.zeros(S, dtype=np.int16)
                idx[: len(rr)] = rr.astype(np.int16)
                dl = np.full(S, -1.0, dtype=np.float32)
                dl[: len(dd)] = dd.astype(np.float32)
                w = idx.reshape(S // 16, 16).T          # [16, S/16] wrap
                iw[:, 8 * col0: 8 * col0 + S // 16] = np.tile(w, (8, 1))
                dw[:, col0: col0 + T] = dl.reshape(T, P).T
                dt_[0, col0 * P: col0 * P + S] = dl
                col0 += T
        assert col0 == Tsum
        idx16.append(iw)
        dlocf.append(dw)
        dloct.append(dt_)
    return idx16, dlocf, dloct, TA.astype(int).tolist(), TB.astype(int).tolist()


def make_weights(cfg, W1, att_src1, att_dst1, W2, att_src2, att_dst2):
    H1, C1 = cfg.H1, cfg.C1
    A1s = np.zeros((cfg.HID, H1), dtype=np.float64)
    A1s[np.arange(cfg.HID), np.arange(cfg.HID) // C1] = att_src1.ravel()
    A1d = np.zeros((cfg.HID, H1), dtype=np.float64)
    A1d[np.arange(cfg.HID), np.arange(cfg.HID) // C1] = att_dst1.ravel()
    # [h 256 | as 8 | ad 8] -> 272 cols
    W1f = np.concatenate([W1, W1 @ A1s, W1 @ A1d], axis=1).astype(np.float32)
    # [g 64 | as2 1 | ad2 1] -> 66 cols
    W2f = np.concatenate([W2, W2 @ att_src2.T, W2 @ att_dst2.T],
                         axis=1).astype(np.float32)
    return W1f, W2f


def bf16(a):
    import ml_dtypes
    return np.asarray(a, dtype=ml_dtypes.bfloat16)


# ---------------------------------------------------------------- device side
GATHER_CHUNK_TILES = 7   # 896 idxs per call; >=1280 fails on HW (ring limit)


def _gather_chunks(TA, TB, table, HALF, R):
    out = []
    for lo, hi, tbl in ((0, TA, table[0:HALF, :]), (TA, TA + TB, table[HALF:R, :])):
        c = lo
        while c < hi:
            e = min(c + GATHER_CHUNK_TILES, hi)
            out.append((c, e, tbl))
            c = e
    return out


def build_kernel(cfg, TA, TB, Tsum):
    nc = bacc.Bacc("TRN2", target_bir_lowering=False, debug=False,
                   num_devices=NCORES)
    IN, HID, OUT, H1, C1 = cfg.IN_CH, cfg.HID, cfg.OUT_CH, cfg.H1, cfg.C1
    U1, U2, TW1, TW2 = cfg.U1, cfg.U2, cfg.TW1, cfg.TW2
    NB, NPAD, R, RB, HALF = cfg.NB, cfg.NPAD, cfg.R, cfg.RB, cfg.HALF
    KI = IN // P   # k-chunks for layer-1 dense
    KH = HID // P  # k-chunks for layer-2 dense
    W1W = U1 + H1  # 272

    # host-pretransposed, per-block interleaved X: [128, RB*KI*128] bf16
    XTI = nc.declare_dram_parameter("XTI", [P, RB * KI * P], BF16, isOutput=False)
    XSI = nc.declare_dram_parameter("XSI", [P, NB * KI * P], BF16, isOutput=False)
    W1F = nc.declare_dram_parameter("W1F", [P, KI, W1W], BF16, isOutput=False)
    W2F = nc.declare_dram_parameter("W2F", [P, KH, U2 + 1], BF16, isOutput=False)
    TDL = nc.declare_dram_parameter("TDL", [P, 9 * Tsum], I16, isOutput=False)
    DLOCT = nc.declare_dram_parameter("DLOCT", [1, Tsum * P], BF16, isOutput=False)
    IOTA = nc.declare_dram_parameter("IOTA", [P, P], BF16, isOutput=False)
    IOTAC = nc.declare_dram_parameter("IOTAC", [P, 1], F32, isOutput=False)
    ONES1 = nc.declare_dram_parameter("ONES1", [1, P], BF16, isOutput=False)
    IDENT = nc.declare_dram_parameter("IDENT", [P, P], F32, isOutput=False)
    B1R = nc.declare_dram_parameter("B1R", [P, HID], F32, isOutput=False)
    B2R = nc.declare_dram_parameter("B2R", [P, OUT], F32, isOutput=False)
    OUTT = nc.declare_dram_parameter("OUTT", [cfg.ND, OUT], F32, isOutput=True)

    with TileContext(nc, num_cores=NCORES) as tc:
        with (
            tc.tile_pool(name="const", bufs=1) as cpool,
            tc.tile_pool(name="dram", bufs=1, space="DRAM") as dram,
        ):
            # resident constants
            iota_sb = cpool.tile([P, P], BF16)
            nc.sync.dma_start(out=iota_sb[:], in_=IOTA[:, :])
            iotac_sb = cpool.tile([P, 1], F32)
            nc.sync.dma_start(out=iotac_sb[:], in_=IOTAC[:, :])
            ones1_sb = cpool.tile([1, P], BF16)
            nc.sync.dma_start(out=ones1_sb[:], in_=ONES1[:, :])
            ident_sb = cpool.tile([P, P], F32)
            nc.sync.dma_start(out=ident_sb[:], in_=IDENT[:, :])
            b1_sb = cpool.tile([P, HID], F32)
            nc.sync.dma_start(out=b1_sb[:], in_=B1R[:, :])
            b2_sb = cpool.tile([P, OUT], F32)
            nc.sync.dma_start(out=b2_sb[:], in_=B2R[:, :])
            w1f_sb = cpool.tile([P, KI, W1W], BF16)
            for k in range(KI):
                nc.sync.dma_start(out=w1f_sb[:, k, :], in_=W1F[:, k, :])
            w2f_sb = cpool.tile([P, KH, U2 + 1], BF16)
            for k in range(KH):
                nc.sync.dma_start(out=w2f_sb[:, k, :], in_=W2F[:, k, :])
            # resident per-own-block attention-dst logits
            ad1_sb = cpool.tile([P, NB, H1], BF16)
            ad2_sb = cpool.tile([P, NB], F32)

            table1 = dram.tile([R, TW1], BF16)
            shard2 = dram.tile([NPAD, TW2], BF16)
            table2 = dram.tile([R, TW2], BF16, addr_space="Shared")

            # ---------------- phase A: replicated layer-1 dense, local table1
            GA = 8   # blocks per batched load/store group
            with (
                tc.tile_pool(name="pa_sb", bufs=3) as sb,
                tc.tile_pool(name="pa_ps", bufs=2, space="PSUM") as ps,
            ):
                assert RB % GA == 0
                for g in range(RB // GA):
                    xt = sb.tile([P, GA, KI, P], BF16, tag="xt")
                    nc.sync.dma_start(
                        out=xt[:],
                        in_=XTI[:, g * GA * KI * P:(g + 1) * GA * KI * P])
                    hrow = sb.tile([P, GA, U1], BF16, tag="hrow")
                    for s in range(GA):
                        ph = ps.tile([P, U1], F32, tag="ph")
                        for k in range(KI):
                            nc.tensor.matmul(out=ph[:], lhsT=xt[:, s, k, :],
                                             rhs=w1f_sb[:, k, 0:U1],
                                             start=(k == 0), stop=(k == KI - 1))
                        nc.scalar.copy(out=hrow[:, s, :], in_=ph[:])
                    # [p, s, c] -> DRAM rows (g*GA+s)*128+p, cols 0:U1
                    nc.scalar.dma_start(
                        out=table1[g * GA * P:(g + 1) * GA * P, 0:U1]
                        .rearrange("(s p) c -> p s c", p=P),
                        in_=hrow[:])

                # phase A': own-shard attention-dst logits (tiny, resident)
                GB = max(g for g in range(1, 9) if NB % g == 0)
                for g in range(NB // GB):
                    xs = sb.tile([P, GB, KI, P], BF16, tag="xs")
                    nc.sync.dma_start(
                        out=xs[:],
                        in_=XSI[:, g * GB * KI * P:(g + 1) * GB * KI * P])
                    for s in range(GB):
                        pa = ps.tile([P, H1], F32, tag="pa")
                        for k in range(KI):
                            nc.tensor.matmul(out=pa[:], lhsT=xs[:, s, k, :],
                                             rhs=w1f_sb[:, k, U1:W1W],
                                             start=(k == 0), stop=(k == KI - 1))
                        nc.scalar.copy(out=ad1_sb[:, g * GB + s, :], in_=pa[:])

            # ---------------- phase B: layer-1 edge aggregation + L2 dense
            with (
                tc.tile_pool(name="pb_he", bufs=3) as p_he,
                tc.tile_pool(name="pb_sb", bufs=2) as sb,
                tc.tile_pool(name="pb_small", bufs=3) as sm,
                tc.tile_pool(name="pb_ps", bufs=2, space="PSUM") as ps,
                tc.tile_pool(name="pb_ps1", bufs=1, space="PSUM") as ps1,
            ):
                icol = 0
                for b in range(NB):
                    T = TA[b] + TB[b]
                    S128 = T * P
                    tdl = sm.tile([P, 9 * T], I16, tag="tdl")
                    nc.sync.dma_start(out=tdl[:],
                                      in_=TDL[:, 9 * icol: 9 * (icol + T)])
                    tidx = tdl[:, 0:8 * T]
                    dloc = tdl[:, 8 * T:9 * T].bitcast(BF16)
                    dlocT = sm.tile([1, S128], BF16, tag="dlocT")
                    nc.scalar.dma_start(
                        out=dlocT[:], in_=DLOCT[:, icol * P: (icol + T) * P])

                    # gather [h | as] rows for this block's edges
                    he = p_he.tile([P, T, TW1], BF16, tag="he")
                    for c0, c1, tbl in _gather_chunks(
                            TA[b], TB[b], table1, HALF, R):
                        nc.gpsimd.dma_gather(
                            he[:, c0:c1, :], tbl, tidx[:, 8 * c0:8 * c1],
                            num_idxs=(c1 - c0) * P, num_idxs_reg=(c1 - c0) * P,
                            elem_size=TW1)

                    # S[e, t, d] one-hot (edge-major) for aggregation
                    S = sb.tile([P, T, P], BF16, tag="S")
                    nc.vector.tensor_tensor(
                        out=S[:], in0=iota_sb[:].unsqueeze(1).to_broadcast([P, T, P]),
                        in1=dloc.unsqueeze(2).to_broadcast([P, T, P]),
                        op=mybir.AluOpType.is_equal)

                    # St[d, e] one-hot (dst-major) via rank-1 PE broadcast
                    St = sb.tile([P, S128], BF16, tag="St")
                    for c0 in range(0, S128, 512):
                        c1 = min(c0 + 512, S128)
                        stb = ps.tile([P, 512], F32, tag="stb")
                        nc.tensor.matmul(out=stb[:, 0:c1 - c0],
                                         lhsT=ones1_sb[:],
                                         rhs=dlocT[:, c0:c1],
                                         start=True, stop=True)
                        nc.vector.tensor_scalar(
                            out=St[:, c0:c1], in0=stb[:, 0:c1 - c0],
                            scalar1=iotac_sb[:, 0:1], scalar2=None,
                            op0=mybir.AluOpType.is_equal)

                    # ad per edge: pad[e, h] = St[:,t]^T-matmul with adb
                    pad = ps1.tile([P, T * H1], F32, tag="pad")
                    for t in range(T):
                        nc.tensor.matmul(
                            out=pad[:, t * H1:(t + 1) * H1],
                            lhsT=St[:, t * P:(t + 1) * P],
                            rhs=ad1_sb[:, b, :],
                            start=True, stop=True)
                    padb = sm.tile([P, T * H1], BF16, tag="padb")
                    nc.scalar.copy(out=padb[:], in_=pad[:])

                    # exp(leaky(as + ad)) -> rhs[:, :, 0:H1]
                    sume = sm.tile([P, T * H1], BF16, tag="sume")
                    nc.vector.tensor_tensor(
                        out=sume[:].rearrange("p (t h) -> p t h", h=H1),
                        in0=he[:, :, HID:U1],
                        in1=padb[:].rearrange("p (t h) -> p t h", h=H1),
                        op=mybir.AluOpType.add)
                    lk = sm.tile([P, T * H1], BF16, tag="lk")
                    nc.vector.scalar_tensor_tensor(
                        out=lk[:], in0=sume[:], scalar=LEAKY, in1=sume[:],
                        op0=mybir.AluOpType.mult, op1=mybir.AluOpType.max)
                    rhs = sb.tile([P, T, H1 + HID], BF16, tag="rhs")
                    nc.scalar.activation(
                        out=rhs[:, :, 0:H1],
                        in_=lk[:].rearrange("p (t h) -> p t h", h=H1),
                        func=mybir.ActivationFunctionType.Exp)
                    # Mw = h * ex (broadcast over the 32 chans of each head)
                    nc.vector.tensor_tensor(
                        out=rhs[:, :, H1:].rearrange("p t (h c) -> p t h c", h=H1),
                        in0=he[:, :, 0:HID].rearrange("p t (h c) -> p t h c", h=H1),
                        in1=rhs[:, :, 0:H1].unsqueeze(3).to_broadcast([P, T, H1, C1]),
                        op=mybir.AluOpType.mult)

                    pm = ps.tile([P, H1 + HID], F32, tag="pm")
                    for t in range(T):
                        nc.tensor.matmul(out=pm[:], lhsT=S[:, t, :], rhs=rhs[:, t, :],
                                         start=(t == 0), stop=(t == T - 1))

                    # normalize + bias + ELU -> h2 block (f32)
                    srec = sm.tile([P, H1], F32, tag="srec")
                    nc.vector.tensor_scalar(
                        out=srec[:], in0=pm[:, 0:H1], scalar1=EPS, scalar2=None,
                        op0=mybir.AluOpType.add)
                    nc.vector.reciprocal(out=srec[:], in_=srec[:])
                    t2 = sm.tile([P, HID], F32, tag="t2")
                    nc.vector.tensor_tensor(
                        out=t2[:].rearrange("p (h c) -> p h c", h=H1),
                        in0=pm[:, H1:].rearrange("p (h c) -> p h c", h=H1),
                        in1=srec[:].unsqueeze(2).to_broadcast([P, H1, C1]),
                        op=mybir.AluOpType.mult)
                    nc.vector.tensor_tensor(out=t2[:], in0=t2[:], in1=b1_sb[:],
                                            op=mybir.AluOpType.add)
                    mm = sm.tile([P, HID], F32, tag="mm")
                    nc.vector.tensor_scalar(out=mm[:], in0=t2[:], scalar1=0.0,
                                            scalar2=None, op0=mybir.AluOpType.min)
                    qq = sm.tile([P, HID], F32, tag="qq")
                    nc.scalar.activation(out=qq[:], in_=mm[:],
                                         func=mybir.ActivationFunctionType.Exp)
                    pp = sm.tile([P, HID], F32, tag="pp")
                    nc.scalar.activation(out=pp[:], in_=t2[:],
                                         func=mybir.ActivationFunctionType.Relu)
                    h2 = sm.tile([P, HID], F32, tag="h2")
                    nc.vector.scalar_tensor_tensor(
                        out=h2[:], in0=qq[:], scalar=-1.0, in1=pp[:],
                        op0=mybir.AluOpType.add, op1=mybir.AluOpType.add)

                    # layer-2 dense for this block: g_ext = h2 @ W2F
                    h2T = sm.tile([P, KH, P], BF16, tag="h2T")
                    for k in range(KH):
                        ptr2 = ps1.tile([P, P], F32, tag="ptr")
                        nc.tensor.transpose(out=ptr2[:], in_=h2[:, k * P:(k + 1) * P],
                                            identity=ident_sb[:])
                        nc.scalar.copy(out=h2T[:, k, :], in_=ptr2[:])
                    pg = ps1.tile([P, U2 + 1], F32, tag="pg")
                    for k in range(KH):
                        nc.tensor.matmul(out=pg[:], lhsT=h2T[:, k, :],
                                         rhs=w2f_sb[:, k, :],
                                         start=(k == 0), stop=(k == KH - 1))
                    gr = sm.tile([P, U2], BF16, tag="gr")
                    nc.scalar.copy(out=gr[:], in_=pg[:, 0:U2])
                    nc.scalar.dma_start(out=shard2[b * P:(b + 1) * P, 0:U2],
                                        in_=gr[:])
                    nc.scalar.copy(out=ad2_sb[:, b:b + 1], in_=pg[:, U2:U2 + 1])
                    icol += T

            nc.gpsimd.collective_compute(
                "AllGather", mybir.AluOpType.bypass,
                replica_groups=[list(range(NCORES))],
                ins=[shard2[:, :].opt()], outs=[table2[:, :].opt()])

            # ---------------- phase C: layer-2 edge aggregation
            with (
                tc.tile_pool(name="pc_ge", bufs=3) as p_ge,
                tc.tile_pool(name="pc_sb", bufs=2) as sb,
                tc.tile_pool(name="pc_small", bufs=3) as sm,
                tc.tile_pool(name="pc_ps", bufs=2, space="PSUM") as ps,
                tc.tile_pool(name="pc_ps1", bufs=1, space="PSUM") as ps1,
            ):
                icol = 0
                for b in range(NB):
                    T = TA[b] + TB[b]
                    S128 = T * P
                    tdl = sm.tile([P, 9 * T], I16, tag="tdl")
                    nc.sync.dma_start(out=tdl[:],
                                      in_=TDL[:, 9 * icol: 9 * (icol + T)])
                    tidx = tdl[:, 0:8 * T]
                    dloc = tdl[:, 8 * T:9 * T].bitcast(BF16)
                    dlocT = sm.tile([1, S128], BF16, tag="dlocT")
                    nc.scalar.dma_start(
                        out=dlocT[:], in_=DLOCT[:, icol * P: (icol + T) * P])

                    ge = p_ge.tile([P, T, TW2], BF16, tag="ge")
                    for c0, c1, tbl in _gather_chunks(
                            TA[b], TB[b], table2, HALF, R):
                        nc.gpsimd.dma_gather(
                            ge[:, c0:c1, :], tbl, tidx[:, 8 * c0:8 * c1],
                            num_idxs=(c1 - c0) * P, num_idxs_reg=(c1 - c0) * P,
                            elem_size=TW2)

                    S = sb.tile([P, T, P], BF16, tag="S")
                    nc.vector.tensor_tensor(
                        out=S[:], in0=iota_sb[:].unsqueeze(1).to_broadcast([P, T, P]),
                        in1=dloc.unsqueeze(2).to_broadcast([P, T, P]),
                        op=mybir.AluOpType.is_equal)

                    St = sb.tile([P, S128], BF16, tag="St")
                    for c0 in range(0, S128, 512):
                        c1 = min(c0 + 512, S128)
                        stb = ps.tile([P, 512], F32, tag="stb")
                        nc.tensor.matmul(out=stb[:, 0:c1 - c0],
                                         lhsT=ones1_sb[:],
                                         rhs=dlocT[:, c0:c1],
                                         start=True, stop=True)
                        nc.vector.tensor_scalar(
                            out=St[:, c0:c1], in0=stb[:, 0:c1 - c0],
                            scalar1=iotac_sb[:, 0:1], scalar2=None,
                            op0=mybir.AluOpType.is_equal)

                    adb2c = sm.tile([P, 1], BF16, tag="adb2c")
                    nc.scalar.copy(out=adb2c[:], in_=ad2_sb[:, b:b + 1])
                    pad2 = ps1.tile([P, T], F32, tag="pad")
                    for t in range(T):
                        nc.tensor.matmul(
                            out=pad2[:, t:t + 1],
                            lhsT=St[:, t * P:(t + 1) * P],
                            rhs=adb2c[:],
                            start=True, stop=True)
                    padb2 = sm.tile([P, T], BF16, tag="padb2")
                    nc.scalar.copy(out=padb2[:], in_=pad2[:])

                    sum2 = sm.tile([P, T], BF16, tag="sum2")
                    nc.vector.tensor_tensor(
                        out=sum2[:], in0=ge[:, :, OUT:U2].squeeze(2),
                        in1=padb2[:], op=mybir.AluOpType.add)
                    lk2 = sm.tile([P, T], BF16, tag="lk2")
                    nc.vector.scalar_tensor_tensor(
                        out=lk2[:], in0=sum2[:], scalar=LEAKY, in1=sum2[:],
                        op0=mybir.AluOpType.mult, op1=mybir.AluOpType.max)
                    rhs2 = sb.tile([P, T, 1 + OUT], BF16, tag="rhs2")
                    nc.scalar.activation(out=rhs2[:, :, 0:1],
                                         in_=lk2[:].unsqueeze(2),
                                         func=mybir.ActivationFunctionType.Exp)
                    nc.vector.tensor_tensor(
                        out=rhs2[:, :, 1:],
                        in0=ge[:, :, 0:OUT],
                        in1=rhs2[:, :, 0:1].to_broadcast([P, T, OUT]),
                        op=mybir.AluOpType.mult)

                    pm2 = ps.tile([P, 1 + OUT], F32, tag="pm")
                    for t in range(T):
                        nc.tensor.matmul(out=pm2[:], lhsT=S[:, t, :],
                                         rhs=rhs2[:, t, :],
                                         start=(t == 0), stop=(t == T - 1))

                    rec2 = sm.tile([P, 1], F32, tag="rec2")
                    nc.vector.tensor_scalar(
                        out=rec2[:], in0=pm2[:, 0:1], scalar1=EPS, scalar2=None,
                        op0=mybir.AluOpType.add)
                    nc.vector.reciprocal(out=rec2[:], in_=rec2[:])
                    ob = sm.tile([P, OUT], F32, tag="ob")
                    nc.vector.scalar_tensor_tensor(
                        out=ob[:], in0=pm2[:, 1:], scalar=rec2[:, 0:1], in1=b2_sb[:],
                        op0=mybir.AluOpType.mult, op1=mybir.AluOpType.add)
                    nrows = min(P, cfg.ND - b * P)
                    nc.scalar.dma_start(out=OUTT[b * P: b * P + nrows, :],
                                        in_=ob[0:nrows, :])
                    icol += T
    return nc


# ---------------------------------------------------------------- entry point
def gat_run(cfg, x, edge_index, W1, att_src1, att_dst1, b1, W2, att_src2,
            att_dst2, b2, trace=False):
    x = np.asarray(x, dtype=np.float32)
    edge_index = np.asarray(edge_index)
    W1f, W2f = make_weights(cfg, np.asarray(W1, np.float64),
                            np.asarray(att_src1, np.float64),
                            np.asarray(att_dst1, np.float64),
                            np.asarray(W2, np.float64),
                            np.asarray(att_src2, np.float64),
                            np.asarray(att_dst2, np.float64))
    idx16, dlocf, dloct, TA, TB = preprocess_graph(
        cfg, edge_index.astype(np.int64))
    Tsum = sum(TA) + sum(TB)

    nc = build_kernel(cfg, TA, TB, Tsum)
    nc.finalize()

    P_, KI = P, cfg.IN_CH // P
    # reordered (core-major padded rows), transposed, per-block interleaved X
    xr = np.zeros((cfg.R, cfg.IN_CH), dtype=np.float32)
    for c in range(NCORES):
        xr[c * cfg.NPAD: c * cfg.NPAD + cfg.ND] = x[c * cfg.ND:(c + 1) * cfg.ND]
    # XTI[p, rb*KI*128 + k*128 + j] = xr[rb*128 + j, k*128 + p]
    xrT = np.ascontiguousarray(
        xr.reshape(cfg.RB, P_, KI, P_).transpose(3, 0, 2, 1))  # [p, rb, k, j]
    xti = bf16(xrT.reshape(P_, cfg.RB * KI * P_))

    w1f_r = bf16(W1f.reshape(KI, P_, cfg.U1 + cfg.H1).transpose(1, 0, 2))
    w2f_r = bf16(W2f.reshape(cfg.HID // P_, P_, cfg.U2 + 1).transpose(1, 0, 2))

    iota = bf16(np.broadcast_to(np.arange(P_, dtype=np.float32), (P_, P_)))
    iotac = np.arange(P_, dtype=np.float32)[:, None].copy()
    ones1 = bf16(np.ones((1, P_), dtype=np.float32))
    ident = np.eye(P_, dtype=np.float32)
    b1r = np.broadcast_to(np.asarray(b1, np.float32), (P_, cfg.HID)).copy()
    b2r = np.broadcast_to(np.asarray(b2, np.float32), (P_, cfg.OUT_CH)).copy()

    in_maps = []
    for c in range(NCORES):
        xsh = xr[c * cfg.NPAD:(c + 1) * cfg.NPAD]
        xshT = np.ascontiguousarray(
            xsh.reshape(cfg.NB, P_, KI, P_).transpose(3, 0, 2, 1))
        xsi = bf16(xshT.reshape(P_, cfg.NB * KI * P_))
        # merged tidx+dloc: per block segment, 8T idx cols then T dloc cols
        tdl = np.zeros((P_, 9 * Tsum), dtype=np.int16)
        dloc_i16 = bf16(dlocf[c]).view(np.int16)
        col = 0
        icol = 0
        for b in range(len(TA)):
            T = TA[b] + TB[b]
            tdl[:, col: col + 8 * T] = idx16[c][:, 8 * icol: 8 * (icol + T)]
            tdl[:, col + 8 * T: col + 9 * T] = dloc_i16[:, icol: icol + T]
            col += 9 * T
            icol += T
        in_maps.append({
            "XTI": xti, "XSI": xsi, "W1F": w1f_r, "W2F": w2f_r,
            "TDL": tdl, "DLOCT": bf16(dloct[c]),
            "IOTA": iota, "IOTAC": iotac, "ONES1": ones1, "IDENT": ident,
            "B1R": b1r, "B2R": b2r,
        })
    res = run_bass_kernel_spmd(nc, in_maps, list(range(NCORES)), trace=trace)
    out = np.concatenate([res.results[c]["OUTT"] for c in range(NCORES)], axis=0)
    return out[:cfg.N], res


def kernel(x, edge_index, W1, att_src1, att_dst1, b1, W2, att_src2, att_dst2,
           b2):
    out, _ = gat_run(CFG_FULL, x, edge_index, W1, att_src1, att_dst1, b1, W2,
                     att_src2, att_dst2, b2)
    return out.astype(np.float32)


# revision 3
# speedup vs baseline: 1.0414x; 1.0414x over previous
"""GAT (2-layer, PyG-style) on 8 Trainium2 NeuronCores — v2.

Design vs baseline:
  - Layer-1 dense phase is REPLICATED on every core (PE is cheap), writing a
    local bf16 table [R, 384] = [h 256 | as 8 | pad], so the 64MB table-1
    AllGather disappears entirely.
  - All gather tables and PE matmuls in bf16 (2x PE rate, ~2x less HBM).
  - Gathers in 7-tile (896-idx) chunks per (dst-block, half) — the largest
    call size the SWDGE descriptor ring tolerates on HW.
  - One-hot "St" (dst-major) built via rank-1 PE broadcast of host-supplied
    dlocT + a single is_equal per 512-col chunk (no per-tile PE transposes).
  - X is fed host-pretransposed+interleaved so the dense phases do straight
    matmuls with no on-device transposes.
  - Node rows are laid out core-major (r = core*NPAD + local) for BOTH
    tables, so one int16 index set serves both layers.
"""
import sys
sys.path.insert(0, "/opt/trn_rl_repo")

import numpy as np
import concourse.bass as bass
import concourse.bacc as bacc
import concourse.mybir as mybir
from concourse.tile import TileContext
from concourse.bass_utils import run_bass_kernel_spmd

F32 = mybir.dt.float32
BF16 = mybir.dt.bfloat16
I16 = mybir.dt.int16

P = 128
NCORES = 8
LEAKY = 0.2
EPS = 1e-16


class Cfg:
    def __init__(self, N, E, IN_CH=256, HID=256, OUT_CH=64, H1=8):
        self.N, self.E = N, E
        self.IN_CH, self.HID, self.OUT_CH, self.H1 = IN_CH, HID, OUT_CH, H1
        self.C1 = HID // H1
        self.ND = N // NCORES                    # dst nodes per core
        self.NB = (self.ND + P - 1) // P         # dst blocks per core
        self.NPAD = self.NB * P                  # padded shard rows
        self.R = NCORES * self.NPAD              # global table rows
        self.RB = NCORES * self.NB               # global table blocks
        self.HALF = self.R // 2                  # int16 gather half split
        assert self.HALF % P == 0
        assert self.HALF < 32768 and (self.R - self.HALF) < 32768
        self.TW1 = 384                           # table1 row width bf16 (264 used)
        self.U1 = HID + H1                       # 264
        self.TW2 = 128                           # table2 row width bf16 (65 used)
        self.U2 = OUT_CH + 1                     # 65 = [g 64 | as2]


CFG_FULL = Cfg(N=50000, E=800000)


# ---------------------------------------------------------------- host side
def _node_row(cfg, n):
    # core-major padded row layout, same for table1 and table2
    return (n // cfg.ND) * cfg.NPAD + (n % cfg.ND)


def preprocess_graph(cfg, edge_index):
    """Per-core wrapped int16 gather indices, dloc (edge-major) and dlocT
    (flat, for the St broadcast), plus shared per-block tile counts."""
    src = np.concatenate([edge_index[0], np.arange(cfg.N, dtype=np.int64)])
    dst = np.concatenate([edge_index[1], np.arange(cfg.N, dtype=np.int64)])
    r_src = _node_row(cfg, src)

    core = dst // cfg.ND
    dst_local = dst - core * cfg.ND
    blk = dst_local // P
    dloc = dst_local % P
    in_a = r_src < cfg.HALF

    NB = cfg.NB
    counts = np.zeros((NCORES, NB, 2), dtype=np.int64)
    np.add.at(counts, (core, blk, (~in_a).astype(np.int64)), 1)
    TA = np.maximum(1, (counts[:, :, 0].max(0) + P - 1) // P)
    TB = np.maximum(1, (counts[:, :, 1].max(0) + P - 1) // P)

    order = np.lexsort((in_a * -1, blk, core))  # by core, block, half (A first)
    rsrc_s, core_s, blk_s, dloc_s, ina_s = (
        r_src[order], core[order], blk[order], dloc[order], in_a[order])

    Tsum = int((TA + TB).sum())
    idx16 = []   # per core: [128, 8 * Tsum] int16
    dlocf = []   # per core: [128, Tsum] bf16-able float
    dloct = []   # per core: [1, Tsum*128] float
    for c in range(NCORES):
        iw = np.zeros((P, 8 * Tsum), dtype=np.int16)
        dw = np.full((P, Tsum), -1.0, dtype=np.float32)
        dt_ = np.full((1, Tsum * P), -1.0, dtype=np.float32)
        csel = core_s == c
        col0 = 0
        for b in range(NB):
            bsel = csel & (blk_s == b)
            for half, T in ((0, int(TA[b])), (1, int(TB[b]))):
                hsel = bsel & (ina_s == (half == 0))
                rr = rsrc_s[hsel] - (0 if half == 0 else cfg.HALF)
                dd = dloc_s[hsel]
                S = T * P
                assert len(rr) <= S
                idx = np.zeros(S, dtype=np.int16)
                idx[: len(rr)] = rr.astype(np.int16)
                dl = np.full(S, -1.0, dtype=np.float32)
                dl[: len(dd)] = dd.astype(np.float32)
                w = idx.reshape(S // 16, 16).T          # [16, S/16] wrap
                iw[:, 8 * col0: 8 * col0 + S // 16] = np.tile(w, (8, 1))
                dw[:, col0: col0 + T] = dl.reshape(T, P).T
                dt_[0, col0 * P: col0 * P + S] = dl
                col0 += T
        assert col0 == Tsum
        idx16.append(iw)
        dlocf.append(dw)
        dloct.append(dt_)
    return idx16, dlocf, dloct, TA.astype(int).tolist(), TB.astype(int).tolist()


def make_weights(cfg, W1, att_src1, att_dst1, W2, att_src2, att_dst2):
    H1, C1 = cfg.H1, cfg.C1
    A1s = np.zeros((cfg.HID, H1), dtype=np.float64)
    A1s[np.arange(cfg.HID), np.arange(cfg.HID) // C1] = att_src1.ravel()
    A1d = np.zeros((cfg.HID, H1), dtype=np.float64)
    A1d[np.arange(cfg.HID), np.arange(cfg.HID) // C1] = att_dst1.ravel()
    # [h 256 | as 8 | ad 8] -> 272 cols
    W1f = np.concatenate([W1, W1 @ A1s, W1 @ A1d], axis=1).astype(np.float32)
    # [g 64 | as2 1 | ad2 1] -> 66 cols
    W2f = np.concatenate([W2, W2 @ att_src2.T, W2 @ att_dst2.T],
                         axis=1).astype(np.float32)
    return W1f, W2f


def bf16(a):
    import ml_dtypes
    return np.asarray(a, dtype=ml_dtypes.bfloat16)


# ---------------------------------------------------------------- device side
GATHER_CHUNK_TILES = 7   # 896 idxs per call; >=1280 fails on HW (ring limit)


def _gather_chunks(TA, TB, table, HALF, R):
    out = []
    for lo, hi, tbl in ((0, TA, table[0:HALF, :]), (TA, TA + TB, table[HALF:R, :])):
        c = lo
        while c < hi:
            e = min(c + GATHER_CHUNK_TILES, hi)
            out.append((c, e, tbl))
            c = e
    return out


def build_kernel(cfg, TA, TB, Tsum):
    nc = bacc.Bacc("TRN2", target_bir_lowering=False, debug=False,
                   num_devices=NCORES)
    IN, HID, OUT, H1, C1 = cfg.IN_CH, cfg.HID, cfg.OUT_CH, cfg.H1, cfg.C1
    U1, U2, TW1, TW2 = cfg.U1, cfg.U2, cfg.TW1, cfg.TW2
    NB, NPAD, R, RB, HALF = cfg.NB, cfg.NPAD, cfg.R, cfg.RB, cfg.HALF
    KI = IN // P   # k-chunks for layer-1 dense
    KH = HID // P  # k-chunks for layer-2 dense
    W1W = U1 + H1  # 272

    # host-pretransposed, per-block interleaved X: [128, RB*KI*128] bf16
    XTI = nc.declare_dram_parameter("XTI", [P, RB * KI * P], BF16, isOutput=False)
    XSI = nc.declare_dram_parameter("XSI", [P, NB * KI * P], BF16, isOutput=False)
    W1F = nc.declare_dram_parameter("W1F", [P, KI, W1W], BF16, isOutput=False)
    W2F = nc.declare_dram_parameter("W2F", [P, KH, U2 + 1], BF16, isOutput=False)
    TDL = nc.declare_dram_parameter("TDL", [P, 9 * Tsum], I16, isOutput=False)
    DLOCT = nc.declare_dram_parameter("DLOCT", [1, Tsum * P], BF16, isOutput=False)
    IOTA = nc.declare_dram_parameter("IOTA", [P, P], BF16, isOutput=False)
    IOTAC = nc.declare_dram_parameter("IOTAC", [P, 1], F32, isOutput=False)
    ONES1 = nc.declare_dram_parameter("ONES1", [1, P], BF16, isOutput=False)
    IDENT = nc.declare_dram_parameter("IDENT", [P, P], F32, isOutput=False)
    B1R = nc.declare_dram_parameter("B1R", [P, HID], F32, isOutput=False)
    B2R = nc.declare_dram_parameter("B2R", [P, OUT], F32, isOutput=False)
    OUTT = nc.declare_dram_parameter("OUTT", [cfg.ND, OUT], F32, isOutput=True)

    with TileContext(nc, num_cores=NCORES) as tc:
        with (
            tc.tile_pool(name="const", bufs=1) as cpool,
            tc.tile_pool(name="dram", bufs=1, space="DRAM") as dram,
        ):
            # resident constants
            iota_sb = cpool.tile([P, P], BF16)
            nc.sync.dma_start(out=iota_sb[:], in_=IOTA[:, :])
            iotac_sb = cpool.tile([P, 1], F32)
            nc.sync.dma_start(out=iotac_sb[:], in_=IOTAC[:, :])
            ones1_sb = cpool.tile([1, P], BF16)
            nc.sync.dma_start(out=ones1_sb[:], in_=ONES1[:, :])
            ident_sb = cpool.tile([P, P], F32)
            nc.sync.dma_start(out=ident_sb[:], in_=IDENT[:, :])
            b1_sb = cpool.tile([P, HID], F32)
            nc.sync.dma_start(out=b1_sb[:], in_=B1R[:, :])
            b2_sb = cpool.tile([P, OUT], F32)
            nc.sync.dma_start(out=b2_sb[:], in_=B2R[:, :])
            w1f_sb = cpool.tile([P, KI, W1W], BF16)
            for k in range(KI):
                nc.sync.dma_start(out=w1f_sb[:, k, :], in_=W1F[:, k, :])
            w2f_sb = cpool.tile([P, KH, U2 + 1], BF16)
            for k in range(KH):
                nc.sync.dma_start(out=w2f_sb[:, k, :], in_=W2F[:, k, :])
            # resident per-own-block attention-dst logits
            ad1_sb = cpool.tile([P, NB, H1], BF16)
            ad2_sb = cpool.tile([P, NB], F32)

            table1 = dram.tile([R, TW1], BF16)
            shard2 = dram.tile([NPAD, TW2], BF16)
            table2 = dram.tile([R, TW2], BF16, addr_space="Shared")

            # ---------------- phase A: replicated layer-1 dense, local table1
            GA = 8   # blocks per batched load/store group
            with (
                tc.tile_pool(name="pa_sb", bufs=3) as sb,
                tc.tile_pool(name="pa_ps", bufs=2, space="PSUM") as ps,
            ):
                assert RB % GA == 0
                for g in range(RB // GA):
                    xt = sb.tile([P, GA, KI, P], BF16, tag="xt")
                    nc.sync.dma_start(
                        out=xt[:],
                        in_=XTI[:, g * GA * KI * P:(g + 1) * GA * KI * P])
                    hrow = sb.tile([P, GA, U1], BF16, tag="hrow")
                    for s in range(GA):
                        ph = ps.tile([P, U1], F32, tag="ph")
                        for k in range(KI):
                            nc.tensor.matmul(out=ph[:], lhsT=xt[:, s, k, :],
                                             rhs=w1f_sb[:, k, 0:U1],
                                             start=(k == 0), stop=(k == KI - 1))
                        nc.scalar.copy(out=hrow[:, s, :], in_=ph[:])
                    # [p, s, c] -> DRAM rows (g*GA+s)*128+p, cols 0:U1
                    nc.scalar.dma_start(
                        out=table1[g * GA * P:(g + 1) * GA * P, 0:U1]
                        .rearrange("(s p) c -> p s c", p=P),
                        in_=hrow[:])

                # phase A': own-shard attention-dst logits (tiny, resident)
                GB = max(g for g in range(1, 9) if NB % g == 0)
                for g in range(NB // GB):
                    xs = sb.tile([P, GB, KI, P], BF16, tag="xs")
                    nc.sync.dma_start(
                        out=xs[:],
                        in_=XSI[:, g * GB * KI * P:(g + 1) * GB * KI * P])
                    for s in range(GB):
                        pa = ps.tile([P, H1], F32, tag="pa")
                        for k in range(KI):
                            nc.tensor.matmul(out=pa[:], lhsT=xs[:, s, k, :],
                                             rhs=w1f_sb[:, k, U1:W1W],
                                             start=(k == 0), stop=(k == KI - 1))
                        nc.scalar.copy(out=ad1_sb[:, g * GB + s, :], in_=pa[:])

            # ---------------- phase B: layer-1 edge aggregation + L2 dense
            with (
                tc.tile_pool(name="pb_he", bufs=3) as p_he,
                tc.tile_pool(name="pb_sb", bufs=2) as sb,
                tc.tile_pool(name="pb_small", bufs=3) as sm,
                tc.tile_pool(name="pb_ps", bufs=2, space="PSUM") as ps,
                tc.tile_pool(name="pb_ps1", bufs=1, space="PSUM") as ps1,
            ):
                icol = 0
                for b in range(NB):
                    T = TA[b] + TB[b]
                    S128 = T * P
                    tdl = sm.tile([P, 9 * T], I16, tag="tdl")
                    nc.sync.dma_start(out=tdl[:],
                                      in_=TDL[:, 9 * icol: 9 * (icol + T)])
                    tidx = tdl[:, 0:8 * T]
                    dloc = tdl[:, 8 * T:9 * T].bitcast(BF16)
                    dlocT = sm.tile([1, S128], BF16, tag="dlocT")
                    nc.scalar.dma_start(
                        out=dlocT[:], in_=DLOCT[:, icol * P: (icol + T) * P])

                    # gather [h | as] rows for this block's edges
                    he = p_he.tile([P, T, TW1], BF16, tag="he")
                    for c0, c1, tbl in _gather_chunks(
                            TA[b], TB[b], table1, HALF, R):
                        nc.gpsimd.dma_gather(
                            he[:, c0:c1, :], tbl, tidx[:, 8 * c0:8 * c1],
                            num_idxs=(c1 - c0) * P, num_idxs_reg=(c1 - c0) * P,
                            elem_size=TW1)

                    # S[e, t, d] one-hot (edge-major) for aggregation
                    S = sb.tile([P, T, P], BF16, tag="S")
                    nc.vector.tensor_tensor(
                        out=S[:], in0=iota_sb[:].unsqueeze(1).to_broadcast([P, T, P]),
                        in1=dloc.unsqueeze(2).to_broadcast([P, T, P]),
                        op=mybir.AluOpType.is_equal)

                    # St[d, e] one-hot (dst-major) via rank-1 PE broadcast
                    St = sb.tile([P, S128], BF16, tag="St")
                    for c0 in range(0, S128, 512):
                        c1 = min(c0 + 512, S128)
                        stb = ps.tile([P, 512], F32, tag="stb")
                        nc.tensor.matmul(out=stb[:, 0:c1 - c0],
                                         lhsT=ones1_sb[:],
                                         rhs=dlocT[:, c0:c1],
                                         start=True, stop=True)
                        nc.vector.tensor_scalar(
                            out=St[:, c0:c1], in0=stb[:, 0:c1 - c0],
                            scalar1=iotac_sb[:, 0:1], scalar2=None,
                            op0=mybir.AluOpType.is_equal)

                    # ad per edge: pad[e, h] = St[:,t]^T-matmul with adb
                    pad = ps1.tile([P, T * H1], F32, tag="pad")
                    for t in range(T):
                        nc.tensor.matmul(
                            out=pad[:, t * H1:(t + 1) * H1],
                            lhsT=St[:, t * P:(t + 1) * P],
                            rhs=ad1_sb[:, b, :],
                            start=True, stop=True)
                    padb = sm.tile([P, T * H1], BF16, tag="padb")
                    nc.scalar.copy(out=padb[:], in_=pad[:])

                    # exp(leaky(as + ad)) -> rhs[:, :, 0:H1]
                    sume = sm.tile([P, T * H1], BF16, tag="sume")
                    nc.vector.tensor_tensor(
                        out=sume[:].rearrange("p (t h) -> p t h", h=H1),
                        in0=he[:, :, HID:U1],
                        in1=padb[:].rearrange("p (t h) -> p t h", h=H1),
                        op=mybir.AluOpType.add)
                    lk = sm.tile([P, T * H1], BF16, tag="lk")
                    nc.vector.scalar_tensor_tensor(
                        out=lk[:], in0=sume[:], scalar=LEAKY, in1=sume[:],
                        op0=mybir.AluOpType.mult, op1=mybir.AluOpType.max)
                    rhs = sb.tile([P, T, H1 + HID], BF16, tag="rhs")
                    nc.scalar.activation(
                        out=rhs[:, :, 0:H1],
                        in_=lk[:].rearrange("p (t h) -> p t h", h=H1),
                        func=mybir.ActivationFunctionType.Exp)
                    # Mw = h * ex (broadcast over the 32 chans of each head)
                    nc.vector.tensor_tensor(
                        out=rhs[:, :, H1:].rearrange("p t (h c) -> p t h c", h=H1),
                        in0=he[:, :, 0:HID].rearrange("p t (h c) -> p t h c", h=H1),
                        in1=rhs[:, :, 0:H1].unsqueeze(3).to_broadcast([P, T, H1, C1]),
                        op=mybir.AluOpType.mult)

                    pm = ps.tile([P, H1 + HID], F32, tag="pm")
                    for t in range(T):
                        nc.tensor.matmul(out=pm[:], lhsT=S[:, t, :], rhs=rhs[:, t, :],
                                         start=(t == 0), stop=(t == T - 1))

                    # normalize + bias + ELU -> h2 block (f32)
                    srec = sm.tile([P, H1], F32, tag="srec")
                    nc.vector.tensor_scalar(
                        out=srec[:], in0=pm[:, 0:H1], scalar1=EPS, scalar2=None,
                        op0=mybir.AluOpType.add)
                    nc.vector.reciprocal(out=srec[:], in_=srec[:])
                    t2 = sm.tile([P, HID], F32, tag="t2")
                    nc.vector.tensor_tensor(
                        out=t2[:].rearrange("p (h c) -> p h c", h=H1),
                        in0=pm[:, H1:].rearrange("p (h c) -> p h c", h=H1),
                        in1=srec[:].unsqueeze(2).to_broadcast([P, H1, C1]),
                        op=mybir.AluOpType.mult)
                    nc.vector.tensor_tensor(out=t2[:], in0=t2[:], in1=b1_sb[:],
                                            op=mybir.AluOpType.add)
                    mm = sm.tile([P, HID], F32, tag="mm")
                    nc.vector.tensor_scalar(out=mm[:], in0=t2[:], scalar1=0.0,
                                            scalar2=None, op0=mybir.AluOpType.min)
                    qq = sm.tile([P, HID], F32, tag="qq")
                    nc.scalar.activation(out=qq[:], in_=mm[:],
                                         func=mybir.ActivationFunctionType.Exp)
                    pp = sm.tile([P, HID], F32, tag="pp")
                    nc.scalar.activation(out=pp[:], in_=t2[:],
                                         func=mybir.ActivationFunctionType.Relu)
                    h2 = sm.tile([P, HID], F32, tag="h2")
                    nc.vector.scalar_tensor_tensor(
                        out=h2[:], in0=qq[:], scalar=-1.0, in1=pp[:],
                        op0=mybir.AluOpType.add, op1=mybir.AluOpType.add)

                    # layer-2 dense for this block: g_ext = h2 @ W2F
                    h2T = sm.tile([P, KH, P], BF16, tag="h2T")
                    for k in range(KH):
                        ptr2 = ps1.tile([P, P], F32, tag="ptr")
                        nc.tensor.transpose(out=ptr2[:], in_=h2[:, k * P:(k + 1) * P],
                                            identity=ident_sb[:])
                        nc.scalar.copy(out=h2T[:, k, :], in_=ptr2[:])
                    pg = ps1.tile([P, U2 + 1], F32, tag="pg")
                    for k in range(KH):
                        nc.tensor.matmul(out=pg[:], lhsT=h2T[:, k, :],
                                         rhs=w2f_sb[:, k, :],
                                         start=(k == 0), stop=(k == KH - 1))
                    gr = sm.tile([P, U2], BF16, tag="gr")
                    nc.scalar.copy(out=gr[:], in_=pg[:, 0:U2])
                    nc.scalar.dma_start(out=shard2[b * P:(b + 1) * P, 0:U2],
                                        in_=gr[:])
                    nc.scalar.copy(out=ad2_sb[:, b:b + 1], in_=pg[:, U2:U2 + 1])
                    icol += T

            nc.gpsimd.collective_compute(
                "AllGather", mybir.AluOpType.bypass,
                replica_groups=[list(range(NCORES))],
                ins=[shard2[:, :].opt()], outs=[table2[:, :].opt()])

            # ---------------- phase C: layer-2 edge aggregation
            with (
                tc.tile_pool(name="pc_ge", bufs=3) as p_ge,
                tc.tile_pool(name="pc_sb", bufs=2) as sb,
                tc.tile_pool(name="pc_small", bufs=3) as sm,
                tc.tile_pool(name="pc_ps", bufs=2, space="PSUM") as ps,
                tc.tile_pool(name="pc_ps1", bufs=1, space="PSUM") as ps1,
            ):
                icol = 0
                for b in range(NB):
                    T = TA[b] + TB[b]
                    S128 = T * P
                    tdl = sm.tile([P, 9 * T], I16, tag="tdl")
                    nc.sync.dma_start(out=tdl[:],
                                      in_=TDL[:, 9 * icol: 9 * (icol + T)])
                    tidx = tdl[:, 0:8 * T]
                    dloc = tdl[:, 8 * T:9 * T].bitcast(BF16)
                    dlocT = sm.tile([1, S128], BF16, tag="dlocT")
                    nc.scalar.dma_start(
                        out=dlocT[:], in_=DLOCT[:, icol * P: (icol + T) * P])

                    ge = p_ge.tile([P, T, TW2], BF16, tag="ge")
                    for c0, c1, tbl in _gather_chunks(
                            TA[b], TB[b], table2, HALF, R):
                        nc.gpsimd.dma_gather(
                            ge[:, c0:c1, :], tbl, tidx[:, 8 * c0:8 * c1],
                            num_idxs=(c1 - c0) * P, num_idxs_reg=(c1 - c0) * P,
                            elem_size=TW2)

                    S = sb.tile([P, T, P], BF16, tag="S")
                    nc.vector.tensor_tensor(
                        out=S[:], in0=iota_sb[:].unsqueeze(1).to_broadcast([P, T, P]),
                        in1=dloc.unsqueeze(2).to_broadcast([P, T, P]),
                        op=mybir.AluOpType.is_equal)

                    St = sb.tile([P, S128], BF16, tag="St")
                    for c0 in range(0, S128, 512):
                        c1 = min(c0 + 512, S128)
                        stb = ps.tile([P, 512], F32, tag="stb")
                        nc.tensor.matmul(out=stb[:, 0:c1 - c0],
                                         lhsT=ones1_sb[:],
                                         rhs=dlocT[:, c0:c1],
                                         start=True, stop=True)
                        nc.vector.tensor_scalar(
                            out=St[:, c0:c1], in0=stb[:, 0:c1 - c0],
                            scalar1=iotac_sb[:, 0:1], scalar2=None,
                            op0=mybir.AluOpType.is_equal)

                    adb2c = sm.tile([P, 1], BF16, tag="adb2c")
                    nc.scalar.copy(out=adb2c[:], in_=ad2_sb[:, b:b + 1])
                    pad2 = ps1.tile([P, T], F32, tag="pad")
                    for t in range(T):
                        nc.tensor.matmul(
                            out=pad2[:, t:t + 1],
                            lhsT=St[:, t * P:(t + 1) * P],
                            rhs=adb2c[:],
                            start=True, stop=True)
                    padb2 = sm.tile([P, T], BF16, tag="padb2")
                    nc.scalar.copy(out=padb2[:], in_=pad2[:])

                    sum2 = sm.tile([P, T], BF16, tag="sum2")
                    nc.vector.tensor_tensor(
                        out=sum2[:], in0=ge[:, :, OUT:U2].squeeze(2),
                        in1=padb2[:], op=mybir.AluOpType.add)
                    lk2 = sm.tile([P, T], BF16, tag="lk2")
                    nc.vector.scalar_tensor_tensor(
                        out=lk2[:], in0=sum2[:], scalar=LEAKY, in1=sum2[:],
                        op0=mybir.AluOpType.mult, op1=mybir.AluOpType.max)
                    rhs2 = sb.tile([P, T, 1 + OUT], BF16, tag="rhs2")
                    nc.scalar.activation(out=rhs2[:, :, 0:1],
                                         in_=lk2[:].unsqueeze(2),
                                         func=mybir.ActivationFunctionType.Exp)
                    nc.vector.tensor_tensor(
                        out=rhs2[:, :, 1:],
                        in0=ge[:, :, 0:OUT],
                        in1=rhs2[:, :, 0:1].to_broadcast([P, T, OUT]),
                        op=mybir.AluOpType.mult)

                    pm2 = ps.tile([P, 1 + OUT], F32, tag="pm")
                    for t in range(T):
                        nc.tensor.matmul(out=pm2[:], lhsT=S[:, t, :],
                                         rhs=rhs2[:, t, :],
                                         start=(t == 0), stop=(t == T - 1))

                    rec2 = sm.tile([P, 1], F32, tag="rec2")
                    nc.vector.tensor_scalar(
                        out=rec2[:], in0=pm2[:, 0:1], scalar1=EPS, scalar2=None,
                        op0=mybir.AluOpType.add)
                    nc.vector.reciprocal(out=rec2[:], in_=rec2[:])
                    ob = sm.tile([P, OUT], F32, tag="ob")
                    nc.vector.scalar_tensor_tensor(
                        out=ob[:], in0=pm2[:, 1:], scalar=rec2[:, 0:1], in1=b2_sb[:],
                        op0=mybir.AluOpType.mult, op1=mybir.AluOpType.add)
                    nrows = min(P, cfg.ND - b * P)
                    nc.scalar.dma_start(out=OUTT[b * P: b * P + nrows, :],
                                        in_=ob[0:nrows, :])
                    icol += T
    return nc


# ---------------------------------------------------------------- entry point
def gat_run(cfg, x, edge_index, W1, att_src1, att_dst1, b1, W2, att_src2,
            att_dst2, b2, trace=False):
    x = np.asarray(x, dtype=np.float32)
    edge_index = np.asarray(edge_index)
    W1f, W2f = make_weights(cfg, np.asarray(W1, np.float64),
                            np.asarray(att_src1, np.float64),
                            np.asarray(att_dst1, np.float64),
                            np.asarray(W2, np.float64),
                            np.asarray(att_src2, np.float64),
                            np.asarray(att_dst2, np.float64))
    idx16, dlocf, dloct, TA, TB = preprocess_graph(
        cfg, edge_index.astype(np.int64))
    Tsum = sum(TA) + sum(TB)

    nc = build_kernel(cfg, TA, TB, Tsum)
    nc.finalize()

    P_, KI = P, cfg.IN_CH // P
    # reordered (core-major padded rows), transposed, per-block interleaved X
    xr = np.zeros((cfg.R, cfg.IN_CH), dtype=np.float32)
    for c in range(NCORES):
        xr[c * cfg.NPAD: c * cfg.NPAD + cfg.ND] = x[c * cfg.ND:(c + 1) * cfg.ND]
    # XTI[p, rb*KI*128 + k*128 + j] = xr[rb*128 + j, k*128 + p]
    xrT = np.ascontiguousarray(
        xr.reshape(cfg.RB, P_, KI, P_).transpose(3, 0, 2, 1))  # [p, rb, k, j]
    xti = bf16(xrT.reshape(P_, cfg.RB * KI * P_))

    w1f_r = bf16(W1f.reshape(KI, P_, cfg.U1 + cfg.H1).transpose(1, 0, 2))
    w2f_r = bf16(W2f.reshape(cfg.HID // P_, P_, cfg.U2 + 1).transpose(1, 0, 2))

    iota = bf16(np.broadcast_to(np.arange(P_, dtype=np.float32), (P_, P_)))
    iotac = np.arange(P_, dtype=np.float32)[:, None].copy()
    ones1 = bf16(np.ones((1, P_), dtype=np.float32))
    ident = np.eye(P_, dtype=np.float32)
    b1r = np.broadcast_to(np.asarray(b1, np.float32), (P_, cfg.HID)).copy()
    b2r = np.broadcast_to(np.asarray(b2, np.float32), (P_, cfg.OUT_CH)).copy()

    in_maps = []
    for c in range(NCORES):
        xsh = xr[c * cfg.NPAD:(c + 1) * cfg.NPAD]
        xshT = np.ascontiguousarray(
            xsh.reshape(cfg.NB, P_, KI, P_).transpose(3, 0, 2, 1))
        xsi = bf16(xshT.reshape(P_, cfg.NB * KI * P_))
        # merged tidx+dloc: per block segment, 8T idx cols then T dloc cols
        tdl = np.zeros((P_, 9 * Tsum), dtype=np.int16)
        dloc_i16 = bf16(dlocf[c]).view(np.int16)
        col = 0
        icol = 0
        for b in range(len(TA)):
            T = TA[b] + TB[b]
            tdl[:, col: col + 8 * T] = idx16[c][:, 8 * icol: 8 * (icol + T)]
            tdl[:, col + 8 * T: col + 9 * T] = dloc_i16[:, icol: icol + T]
            col += 9 * T
            icol += T
        in_maps.append({
            "XTI": xti, "XSI": xsi, "W1F": w1f_r, "W2F": w2f_r,
            "TDL": tdl, "DLOCT": bf16(dloct[c]),
            "IOTA": iota, "IOTAC": iotac, "ONES1": ones1, "IDENT": ident,
            "B1R": b1r, "B2R": b2r,
        })
    res = run_bass_kernel_spmd(nc, in_maps, list(range(NCORES)), trace=trace)
    out = np.concatenate([res.results[c]["OUTT"] for c in range(NCORES)], axis=0)
    return out[:cfg.N], res


def kernel(x, edge_index, W1, att_src1, att_dst1, b1, W2, att_src2, att_dst2,
           b2):
    out, _ = gat_run(CFG_FULL, x, edge_index, W1, att_src1, att_dst1, b1, W2,
                     att_src2, att_dst2, b2)
    return out.astype(np.float32)
